# revision 32
# baseline (speedup 1.0000x reference)
"""Trainium2 Bass kernel for a 2-layer GCN (PyG GCNConv x2 with self-loops).

Reference computation (N=100000 nodes, E=1600000 edges, f32):
    row, col = add_self_loops(edge_index)
    deg  = in-degree over col (incl. self loops); dis = rsqrt(deg)
    norm = dis[row] * dis[col]
    A_hat X = segment_sum(X[row] * norm, col)          # normalized aggregation
    h   = relu(A_hat X @ W1 + b1)                      # aggregate-then-transform
    out = (A_hat h) @ W2 + b2

Key algebraic identity: segment_sum((X W)[row]*norm, col) ==
segment_sum(X[row]*norm, col) @ W, so aggregation happens in the *input*
feature dim (64 / 128) which minimizes gather traffic.

Distribution over 8 NeuronCores: destination-node sharding (12500 nodes
per core).  Layer-1 aggregation / h / AllGather are pipelined per
node-quarter so the collectives hide behind layer-1 gather work; the
all-gathered h is copied from Shared DRAM to local DRAM before layer-2
gathers (Shared-space gather reads are ~2.7x slower than local ones).

Per-core aggregation (no scatter / no races):
  - edges (incl. self-loops) are bucketed host-side by (source-chunk k,
    dest-window w); windows are 128 destinations wide.
  - the device gathers source rows with gpsimd.dma_gather (G=1024 edges
    per call, 4 SWDGE queues round-robin, single_packet mode).
  - the symmetric norm dis[src]*dis[dst] is FACTORIZED out of the edge
    stream: dis[src] is pre-multiplied into the gathered rows (host-side
    for x, via an extra scalar-engine scale for h), dis[dst] is applied
    by the per-window output stages (activation per-partition scale +
    bias matmul against a 1/dis row instead of ones).  The selection
    tensor P[e, d] = (dest_e == d) is therefore a pure one-hot built
    with a SINGLE fp16 vector is_equal; each 128-edge chunk accumulates
    PSUM[feat, dest] += msg_chunk.T @ P on the tensor engine.
  - x is staged as fp16 row pairs xd[i] = [x~[i], x~[i+1]] so the
    256B-aligned gather element carries a 64-feature fp16 row (the pair
    tail is ballast) and every matmul runs at full fp16 PE rate.

All cores run one identical program (SPMD); per-(k,w) chunk counts are
equalized across cores host-side with padding edges whose dst=-1 makes
their one-hot rows all-zero.

Perf notes (HW-measured): the kernel is bound by the serialized
gpsimd dma_gather chain (~4us per 1024-edge call: ~2.5us Q7 descriptor
generation + non-overlapped SDMA drain/dispatch overhead).  Vector /
tensor / scalar engines all have slack.  Measured dead ends: per-slot
AP-scalar tensor_scalar (1.9us/op fixed), single_packet=0 (+13%),
larger descriptor rings (no change).
"""

import os
import sys

import numpy as np

for _p in ("/opt/trn_rl_repo", "/root/.axon_site/_ro/trn_rl_repo"):
    if os.path.isdir(_p) and _p not in sys.path:
        sys.path.insert(0, _p)

# ----------------------------------------------------------------------------
# Problem constants (hardcoded per the harness contract)
# ----------------------------------------------------------------------------
N = 100000          # nodes
NC = 8              # cores
NS = N // NC        # 12500 dest nodes per core
D0, D1, D2 = 64, 128, 256
WIN = 128           # dest window width
NW = (NS + WIN - 1) // WIN          # 98 windows per core
NSRC = 4            # source chunks (int16 gather index limit)
SC1 = N // NSRC     # 25000 rows per layer-1 source chunk
Q4 = NS // NSRC     # 3125 rows per all-gather chunk slice
G = int(os.environ.get("GCN_G", "1024"))   # edges per dma_gather call;
                    # hard ucode cap: calls >1024 descriptors deadlock on
                    # hardware even with a larger dynamic_dma_scratch_size
NQ = 4              # SWDGE queues (ucode max); gather calls round-robin
_SINGLE_PACKET = os.environ.get("GCN_SINGLE_PACKET", "1") == "1"
# prepare_only + trigger_dma measured SLOWER on hardware (the ring reclaim
# blocks the next prep through the previous transfer anyway, plus ~2x
# per-call overhead) and its completion-sem protocol mis-syncs; keep off.
_PREP = os.environ.get("GCN_PREP", "0") == "1"

# AllGather quarter boundaries in dest windows: AG q needs h rows
# [q*3125, (q+1)*3125) -> all windows < ceil((q+1)*3125/128) written.
WB = [0, 25, 49, 74, 98]
# Sub-groups bound the number of live PSUM window accumulators.  PSUM
# accumulation groups are zero-region (2KB bank) granular, so each open
# window needs its own bank: 5 window banks + 2 h-stage banks < 8.
SUBS = []
for _q in range(4):
    _a, _b = WB[_q], WB[_q + 1]
    for _s in range(_a, _b, 5):
        SUBS.append((_s, min(_s + 5, _b)))


# ----------------------------------------------------------------------------
# Host-side preprocessing: sharding + edge bucketing
# ----------------------------------------------------------------------------
def _plan_layer(src_all, dloc_all, core_all, src_chunk_of, src_idx_of):
    """Bucket each core's edges by (source-chunk k, dest-window w), pad each
    bucket to a multiple of 128 edges AND to identical chunk counts across
    all cores (SPMD requires one program).

    The symmetric norm dis[src]*dis[dst] is factorized out of the edge
    stream entirely: dis[src] is folded into the gathered source rows
    (x / h pre-scaled), dis[dst] into the per-window output stages.  Each
    edge therefore only carries (idx, dst); P is a pure one-hot.

    Returns (meta, per_core_arrays):
      meta = {"Tk": [...], "segs": [[(w, n_chunks), ...] per k]}
      per_core_arrays[c] = {"idx": [...], "dst": [...]}
    """
    NWl = NW
    counts = np.zeros((NC, NSRC, NWl), dtype=np.int64)
    per_core = []
    for c in range(NC):
        sel = core_all == c
        src = src_all[sel]
        dloc = dloc_all[sel]
        k = src_chunk_of(src)
        w = dloc // WIN
        idxl = src_idx_of(src)
        order = np.lexsort((w, k))
        k, w, idxl, dloc = k[order], w[order], idxl[order], dloc[order]
        key = k * NWl + w
        counts[c] = np.bincount(key, minlength=NSRC * NWl).reshape(NSRC, NWl)
        per_core.append((k, w, idxl, dloc, key))

    nch = (counts.max(axis=0) + 127) // 128          # [NSRC, NW] chunks per bucket
    Tk = (nch.sum(axis=1) * 128).astype(np.int64)    # padded edges per chunk k
    segs = [[(int(w), int(nch[k, w])) for w in range(NWl) if nch[k, w] > 0]
            for k in range(NSRC)]

    base = np.zeros((NSRC, NWl), dtype=np.int64)
    for k in range(NSRC):
        base[k] = np.concatenate(([0], np.cumsum(nch[k] * 128)[:-1]))

    out = []
    for c in range(NC):
        k, w, idxl, dloc, key = per_core[c]
        cnt = counts[c].reshape(-1)
        starts = np.concatenate(([0], np.cumsum(cnt)[:-1]))
        pos_in_bucket = np.arange(len(key)) - starts[key]
        tgt = base.reshape(-1)[key] + pos_in_bucket   # position inside k-stream
        arrs = {"idx": [], "dst": []}
        for kk in range(NSRC):
            T = int(Tk[kk])
            idx16 = np.zeros(T, dtype=np.int16)
            dwf = np.full(T, -1.0, dtype=np.float32)
            m = k == kk
            t = tgt[m]
            idx16[t] = idxl[m].astype(np.int16)
            dwf[t] = (dloc[m] - (w[m] * WIN)).astype(np.float32)
            # device layouts: idx wraps by 16 (replicated to 128 partitions
            # for the 8 gpsimd cores), dst wraps by 128
            arrs["idx"].append(np.ascontiguousarray(
                np.tile(idx16.reshape(T // 16, 16).T, (8, 1))))
            arrs["dst"].append(np.ascontiguousarray(
                dwf.reshape(T // 128, 128).T.astype(np.float16)))
        out.append(arrs)
    return {"Tk": [int(t) for t in Tk], "segs": segs}, out


def _preprocess(x, edge_index, W1, b1, W2, b2):
    row = np.asarray(edge_index[0], dtype=np.int64)
    col = np.asarray(edge_index[1], dtype=np.int64)
    deg = (np.bincount(col, minlength=N) + 1).astype(np.float32)  # + self loop
    dis = (1.0 / np.sqrt(deg)).astype(np.float32)

    loop = np.arange(N, dtype=np.int64)
    rows = np.concatenate([row, loop])
    cols = np.concatenate([col, loop])
    core = (cols // NS).astype(np.int64)
    dloc = cols - core * NS

    # L1 source chunks are INTERLEAVED (src % NSRC) rather than contiguous:
    # a core's self-loop sources all fall in one contiguous chunk, which
    # would skew the cross-core bucket maxima and inflate SPMD padding.
    meta1, arrs1 = _plan_layer(
        rows, dloc, core,
        src_chunk_of=lambda s: s % NSRC,
        src_idx_of=lambda s: s // NSRC,
    )
    meta2, arrs2 = _plan_layer(
        rows, dloc, core,
        src_chunk_of=lambda s: (s % NS) // Q4,
        src_idx_of=lambda s: (s // NS) * Q4 + (s % NS) % Q4,
    )

    x = np.asarray(x, dtype=np.float32)
    # fp16 pair-duplicated, dis[src]-prescaled x: xd[i] = [x~[i], x~[i+1]];
    # the gather element is 128 fp16 = 256 bytes, of which the first 64 are
    # the row we want.
    xf16 = (x * dis[:, None]).astype(np.float16)
    xd = np.zeros((N, 2 * D0), dtype=np.float16)
    xd[:, :D0] = xf16
    xd[:-1, D0:] = xf16[1:]

    shared = {
        "xd": np.ascontiguousarray(xd),
        "W1": np.ascontiguousarray(np.asarray(W1, dtype=np.float32)),
        "b1": np.ascontiguousarray(
            np.asarray(b1, dtype=np.float32).reshape(1, D1)),
        "W2": np.ascontiguousarray(np.asarray(W2, dtype=np.float32)),
        "b2": np.ascontiguousarray(
            np.asarray(b2, dtype=np.float32).reshape(1, D2)),
    }
    in_maps = []
    for c in range(NC):
        m = dict(shared)
        # Per-core dis layouts for the dst-side factor (shard padded to
        # NW*WIN = 12544 rows):
        #   dis_act[p, w] = dis[c*NS + w*128 + p]  (activation per-part scale)
        #   disinv[0, j]  = 1/dis[c*NS + j]        (bias-matmul stationary row)
        dc = np.ones(NW * WIN, dtype=np.float32)
        dc[:NS] = dis[c * NS:(c + 1) * NS]
        di = np.zeros(NW * WIN, dtype=np.float32)
        di[:NS] = 1.0 / dis[c * NS:(c + 1) * NS]
        m["dis_act"] = np.ascontiguousarray(dc.reshape(NW, WIN).T)
        m["disinv"] = np.ascontiguousarray(di.reshape(1, NW * WIN))
        for kk in range(NSRC):
            m[f"idx1_{kk}"] = arrs1[c]["idx"][kk]
            m[f"dst1_{kk}"] = arrs1[c]["dst"][kk]
            m[f"idx2_{kk}"] = arrs2[c]["idx"][kk]
            m[f"dst2_{kk}"] = arrs2[c]["dst"][kk]
        in_maps.append(m)
    return meta1, meta2, in_maps


# ----------------------------------------------------------------------------
# Device program
# ----------------------------------------------------------------------------
def _build(meta1, meta2, debug=False, stage="full"):
    from contextlib import ExitStack

    import concourse.bacc as bacc
    import concourse.bass as bass
    import concourse.mybir as mybir
    import concourse.tile as tile

    f32, f16, i16 = mybir.dt.float32, mybir.dt.float16, mybir.dt.int16
    GC = G // 128

    nc = bacc.Bacc("TRN2", target_bir_lowering=False, debug=debug,
                   num_devices=NC, num_swdge_queues=NQ,
                   dynamic_dma_scratch_size=int(
                       os.environ.get("GCN_SCRATCH", str(16 * G))))

    xd_d = nc.dram_tensor("xd", [N, 2 * D0], f16, kind="ExternalInput")
    w1_d = nc.dram_tensor("W1", [D0, D1], f32, kind="ExternalInput")
    b1_d = nc.dram_tensor("b1", [1, D1], f32, kind="ExternalInput")
    w2_d = nc.dram_tensor("W2", [D1, D2], f32, kind="ExternalInput")
    b2_d = nc.dram_tensor("b2", [1, D2], f32, kind="ExternalInput")
    dis_act_d = nc.dram_tensor("dis_act", [WIN, NW], f32, kind="ExternalInput")
    disinv_d = nc.dram_tensor("disinv", [1, NW * WIN], f32, kind="ExternalInput")

    idx1_d, dst1_d, idx2_d, dst2_d = [], [], [], []
    for k in range(NSRC):
        T1, T2 = meta1["Tk"][k], meta2["Tk"][k]
        idx1_d.append(nc.dram_tensor(f"idx1_{k}", [128, T1 // 16], i16, kind="ExternalInput"))
        dst1_d.append(nc.dram_tensor(f"dst1_{k}", [128, T1 // 128], f16, kind="ExternalInput"))
        idx2_d.append(nc.dram_tensor(f"idx2_{k}", [128, T2 // 16], i16, kind="ExternalInput"))
        dst2_d.append(nc.dram_tensor(f"dst2_{k}", [128, T2 // 128], f16, kind="ExternalInput"))

    h_own = nc.dram_tensor("h_own", [NS, D1], f16, kind="Internal")
    hf = [nc.dram_tensor(f"hf{q}", [NC * Q4, D1], f16, kind="Internal",
                         addr_space="Shared") for q in range(NSRC)]
    hl = [nc.dram_tensor(f"hl{q}", [NC * Q4, D1], f16, kind="Internal")
          for q in range(NSRC)]
    if stage == "A":
        out_d = nc.dram_tensor("out", [D0, NW * WIN], f32, kind="ExternalOutput")
    elif stage == "AB":
        out_d = nc.dram_tensor("out", [NS, D1], f16, kind="ExternalOutput")
    elif stage == "ABC":
        out_d = nc.dram_tensor("out", [NC * Q4, D1], f16, kind="ExternalOutput")
    elif stage == "AD":
        out_d = nc.dram_tensor("out", [D1, NW * WIN], f32, kind="ExternalOutput")
    else:
        out_d = nc.dram_tensor("out", [NS, D2], f32, kind="ExternalOutput")
    acc2_dump = (nc.dram_tensor("acc2dump", [D1, NW * WIN], f32,
                                kind="ExternalOutput")
                 if stage == "full+dump" else None)
    if stage == "full+dump":
        stage = "full"

    # per-bucket chunk counts / prefix offsets per k-stream
    nch1 = np.zeros((NSRC, NW), dtype=np.int64)
    for k in range(NSRC):
        for (w, n) in meta1["segs"][k]:
            nch1[k][w] = n
    pre1 = np.zeros((NSRC, NW + 1), dtype=np.int64)
    for k in range(NSRC):
        pre1[k][1:] = np.cumsum(nch1[k])
    # first/last source-chunk contributing to each window (PSUM start/stop)
    fk = [min(k for k in range(NSRC) if nch1[k][w] > 0) for w in range(NW)]
    lk = [max(k for k in range(NSRC) if nch1[k][w] > 0) for w in range(NW)]

    qsems = [nc.alloc_semaphore(f"gq{i}") for i in range(NQ)]
    qstate = {"n": 0, "cum": [0] * NQ}
    drain_hl_hook = [lambda: None]

    def gather_call(gt_ap, src_ap, idx_ap, mlen, elem):
        """Issue one gather call.  With _PREP, descriptor generation is
        decoupled from the transfer (prepare_only + trigger) so the Pool
        engine never blocks on the DMA; the data-completion sync is the
        caller's job: we emit an explicit PE wait_ge on the queue's DMA
        semaphore right before the first consuming matmul (Tile's
        lane-sem waits are pre-bumped away by the framework)."""
        q = qstate["n"] % NQ
        qstate["n"] += 1
        drain_hl_hook[0]()
        if _PREP:
            nc.gpsimd.dma_gather(
                gt_ap, src_ap, idx_ap, mlen, mlen, elem,
                elem_step=src_ap.ap[0][0], queue_num=q,
                single_packet=_SINGLE_PACKET,
                prepare_only=True, sem=qsems[q])
            nc.gpsimd.trigger_dma(count=None, queue_num=q)
            qstate["cum"][q] += 16
            nc.tensor.wait_ge(qsems[q], qstate["cum"][q])
        else:
            nc.gpsimd.dma_gather(
                gt_ap, src_ap, idx_ap, mlen, mlen, elem,
                elem_step=src_ap.ap[0][0], queue_num=q,
                single_packet=_SINGLE_PACKET)

    with tile.TileContext(nc) as tc:
        with ExitStack() as top:
            const = top.enter_context(tc.tile_pool(name="const", bufs=1))
            w1_t = const.tile([D0, D1], f32)
            nc.sync.dma_start(w1_t[:], w1_d[:])
            b1_t = const.tile([1, D1], f32)
            nc.sync.dma_start(b1_t[:], b1_d[:])
            w2_t = const.tile([D1, D2], f32)
            nc.sync.dma_start(w2_t[:], w2_d[:])
            b2_t = const.tile([1, D2], f32)
            nc.sync.dma_start(b2_t[:], b2_d[:])
            dis_act_t = const.tile([WIN, NW], f32)
            nc.sync.dma_start(dis_act_t[:], dis_act_d[:])
            disinv_t = const.tile([1, NW * WIN], f32)
            nc.sync.dma_start(disinv_t[:], disinv_d[:])
            iota_t = const.tile([128, GC, WIN], f16)
            nc.gpsimd.iota(iota_t[:], pattern=[[0, GC], [1, WIN]], base=0,
                           channel_multiplier=0,
                           allow_small_or_imprecise_dtypes=True)

            accp = top.enter_context(tc.tile_pool(name="acc", bufs=1))
            acc1 = accp.tile([D0, NW * WIN], f32)
            nc.vector.memset(acc1[:], 0.0)
            acc2 = accp.tile([D1, NW * WIN], f32)
            nc.vector.memset(acc2[:], 0.0)

            xb = xd_d[:]
            x_aps = [bass.AP(xb.tensor, k * 2 * D0,
                             [[NSRC * 2 * D0, SC1], [1, 2 * D0]])
                     for k in range(NSRC)]

            def bcast(col_slice, mc):
                return bass.AP(col_slice.tensor, col_slice.offset,
                               [list(col_slice.ap[0]), [1, mc], [0, WIN]])

            # Deferred AllGather emission: the collective trigger sits in
            # the Pool queue and head-of-line blocks subsequent gather
            # calls while waiting on the h DMAs; emitting it AFTER the
            # next phase's first gather call hides that wait.
            pending_ag = []
            pending_hl = []
            HL_PIECES = 8

            def flush_ag():
                while pending_ag:
                    qq = pending_ag.pop(0)
                    nc.gpsimd.collective_compute(
                        "AllGather", mybir.AluOpType.bypass,
                        replica_groups=[list(range(NC))],
                        ins=[h_own[qq * Q4:(qq + 1) * Q4, :]],
                        outs=[hf[qq][:, :]],
                    )
                    for pc in range(HL_PIECES):
                        pending_hl.append((qq, pc))

            # The 6.4MB hf->hl staging copies monopolize the SDMA engines
            # when issued as one DMA (the big post-AllGather gather stalls in
            # the profile).  Instead they are chunked and trickled out, one
            # piece per subsequent gather call, so gather packets interleave.
            def drain_hl():
                if pending_hl:
                    qq, pc = pending_hl.pop(0)
                    r0 = pc * (NC * Q4 // HL_PIECES)
                    r1 = r0 + NC * Q4 // HL_PIECES
                    nc.sync.dma_start(hl[qq][r0:r1, :], hf[qq][r0:r1, :])

            drain_hl_hook[0] = drain_hl

            # ---------------- Layer 1 + h + AllGather, per quarter --------
            with ExitStack() as l1s:
                mp = l1s.enter_context(tc.tile_pool(name="meta1", bufs=6))
                gp = l1s.enter_context(tc.tile_pool(name="g1", bufs=8))
                pp = l1s.enter_context(tc.tile_pool(name="p1", bufs=6))
                psp1 = l1s.enter_context(
                    tc.tile_pool(name="ps1", bufs=6, space="PSUM"))
                psb = l1s.enter_context(
                    tc.tile_pool(name="psb", bufs=2, space="PSUM"))
                hp = l1s.enter_context(tc.tile_pool(name="hb", bufs=4))

                for qi in range(4):
                    g0, g1 = WB[qi], WB[qi + 1]
                    for k in range(NSRC):
                        c0, c1 = int(pre1[k][g0]), int(pre1[k][g1])
                        total = c1 - c0
                        if total == 0:
                            continue
                        idx_t = mp.tile([128, total * 8], i16, tag="idx")
                        dst_t = mp.tile([128, total], f16, tag="dst")
                        NPC = 4
                        cw = (total + NPC - 1) // NPC
                        for pc in range(NPC):
                            a, b = pc * cw, min((pc + 1) * cw, total)
                            if a >= b:
                                break
                            nc.sync.dma_start(
                                idx_t[:, a * 8:b * 8],
                                idx1_d[k][:, (c0 + a) * 8:(c0 + b) * 8])
                            nc.sync.dma_start(dst_t[:, a:b],
                                              dst1_d[k][:, c0 + a:c0 + b])
                        jj = 0
                        gt = None
                        P8 = None
                        for w in range(g0, g1):
                            nchk = int(nch1[k][w])
                            if nchk == 0:
                                continue
                            ps = psp1.tile([D0, WIN], f32, tag="ps1")
                            for j in range(nchk):
                                t, slot = divmod(jj, GC)
                                if slot == 0:
                                    mc = min(GC, total - t * GC)
                                    gt = gp.tile([128, GC, 2 * D0], f16,
                                                 tag="gt")
                                    gather_call(
                                        gt[:, :mc, :], x_aps[k],
                                        idx_t[:, t * GC * 8:
                                              t * GC * 8 + mc * 8],
                                        mc * 128, 2 * D0)
                                    flush_ag()
                                    P8 = pp.tile([128, GC, WIN], f16,
                                                 tag="P")
                                    nc.vector.tensor_tensor(
                                        P8[:, :mc, :], iota_t[:, :mc, :],
                                        bcast(dst_t[:, t * GC:t * GC + mc], mc),
                                        mybir.AluOpType.is_equal)
                                nc.tensor.matmul(
                                    ps[:], gt[:, slot, 0:D0],
                                    P8[:, slot, :],
                                    start=(j == 0), stop=(j == nchk - 1))
                                jj += 1
                            nc.vector.tensor_tensor(
                                acc1[:, w * WIN:(w + 1) * WIN],
                                acc1[:, w * WIN:(w + 1) * WIN], ps[:],
                                mybir.AluOpType.add)

                    # ---- h~ = dis * relu(dis*(acc1.T @ W1) + b1) ----
                    # ps = acc1.T@W1 + (1/dis)*b1; relu(ps*dis) applies the
                    # dst-side dis INSIDE the relu, the extra mul applies the
                    # src-side dis for layer-2 gathers.
                    for w in range(g0, g1):
                        M = min(WIN, NS - w * WIN)
                        ps = psb.tile([M, D1], f32, tag="psb")
                        nc.tensor.matmul(ps[:], acc1[:, w * WIN:w * WIN + M],
                                         w1_t[:], start=True, stop=False)
                        nc.tensor.matmul(
                            ps[:], disinv_t[:, w * WIN:w * WIN + M], b1_t[:],
                            start=False, stop=True)
                        ht = hp.tile([M, D1], f16, tag="ht")
                        nc.scalar.activation(ht[:], ps[:],
                                             mybir.ActivationFunctionType.Relu,
                                             scale=dis_act_t[0:M, w:w + 1])
                        ht2 = hp.tile([M, D1], f16, tag="ht2")
                        nc.scalar.mul(ht2[:], ht[:],
                                      dis_act_t[0:M, w:w + 1])
                        nc.sync.dma_start(h_own[w * WIN:w * WIN + M, :], ht2[:])

                    pending_ag.append(qi)

            if stage in ("A", "AB", "ABC"):
                flush_ag()
                while pending_hl:
                    drain_hl()
            if stage == "A":
                nc.sync.dma_start(out_d[:], acc1[:])
            elif stage == "AB":
                nc.sync.dma_start(out_d[:], h_own[:])
            elif stage == "ABC":
                nc.sync.dma_start(out_d[:], hl[0][:])

            # ------- Layer 2 (+ folded out-stage on the last k pass) -------
            with ExitStack() as l2s:
                if stage in ("A", "AB", "ABC"):
                    meta2 = {"Tk": [0] * NSRC, "segs": [[] for _ in range(NSRC)]}
                nch2 = np.zeros((NSRC, NW), dtype=np.int64)
                for k in range(NSRC):
                    for (w, n) in meta2["segs"][k]:
                        nch2[k][w] = n
                mp2 = l2s.enter_context(tc.tile_pool(name="meta2", bufs=2))
                gp2 = l2s.enter_context(tc.tile_pool(name="g2", bufs=8))
                pp2 = l2s.enter_context(tc.tile_pool(name="p2", bufs=6))
                psp = l2s.enter_context(
                    tc.tile_pool(name="ps2", bufs=6, space="PSUM"))
                op = l2s.enter_context(tc.tile_pool(name="ob", bufs=4))
                pso = l2s.enter_context(
                    tc.tile_pool(name="pso", bufs=2, space="PSUM"))

                for k in range(NSRC):
                    Tk = meta2["Tk"][k]
                    last_k = k == NSRC - 1
                    if Tk > 0:
                        total = Tk // 128
                        idx_t = mp2.tile([128, Tk // 16], i16, tag="idx2")
                        dst_t = mp2.tile([128, total], f16, tag="dst2")
                        NPC = 8
                        cw = (total + NPC - 1) // NPC
                        for pc in range(NPC):
                            a, b = pc * cw, min((pc + 1) * cw, total)
                            if a >= b:
                                break
                            nc.sync.dma_start(idx_t[:, a * 8:b * 8],
                                              idx2_d[k][:, a * 8:b * 8])
                            nc.sync.dma_start(dst_t[:, a:b],
                                              dst2_d[k][:, a:b])
                        hl_ap = hl[k][:]
                        jj = 0
                        gt = None
                        P8 = None
                    for w in range(NW):
                        nchk = int(nch2[k][w]) if Tk > 0 else 0
                        if nchk > 0:
                            ps = psp.tile([D1, WIN], f32, tag="ps2")
                            for j in range(nchk):
                                t, slot = divmod(jj, GC)
                                if slot == 0:
                                    mc = min(GC, total - t * GC)
                                    gt = gp2.tile([128, GC, D1], f16,
                                                  tag="gt2")
                                    gather_call(
                                        gt[:, :mc, :], hl_ap,
                                        idx_t[:, t * GC * 8:t * GC * 8 + mc * 8],
                                        mc * 128, D1)
                                    flush_ag()
                                    P8 = pp2.tile([128, GC, WIN], f16,
                                                  tag="P2")
                                    nc.vector.tensor_tensor(
                                        P8[:, :mc, :], iota_t[:, :mc, :],
                                        bcast(dst_t[:, t * GC:t * GC + mc], mc),
                                        mybir.AluOpType.is_equal)
                                nc.tensor.matmul(ps[:], gt[:, slot, :],
                                                 P8[:, slot, :],
                                                 start=(j == 0),
                                                 stop=(j == nchk - 1))
                                jj += 1
                            nc.vector.tensor_tensor(
                                acc2[:, w * WIN:(w + 1) * WIN],
                                acc2[:, w * WIN:(w + 1) * WIN], ps[:],
                                mybir.AluOpType.add)
                        if last_k and stage == "full":
                            # acc2[:, w] is final: out = dis*(acc2.T@W2) + b2
                            # emitted as dis*(acc2.T@W2 + (1/dis)*b2).
                            M = min(WIN, NS - w * WIN)
                            pso_t = pso.tile([M, D2], f32, tag="pso")
                            nc.tensor.matmul(
                                pso_t[:], acc2[:, w * WIN:w * WIN + M],
                                w2_t[:], start=True, stop=False)
                            nc.tensor.matmul(
                                pso_t[:], disinv_t[:, w * WIN:w * WIN + M],
                                b2_t[:], start=False, stop=True)
                            ot = op.tile([M, D2], f32, tag="ot")
                            nc.scalar.mul(ot[:], pso_t[:],
                                          dis_act_t[0:M, w:w + 1])
                            nc.sync.dma_start(
                                out_d[w * WIN:w * WIN + M, :], ot[:])

            if stage == "AD":
                nc.sync.dma_start(out_d[:], acc2[:])
            if acc2_dump is not None:
                nc.sync.dma_start(acc2_dump[:], acc2[:])

    nc.compile()
    return nc


# ----------------------------------------------------------------------------
# Entry point
# ----------------------------------------------------------------------------
def _ensure_axon_hooks_module():
    """bass_utils hard-imports antenv.axon_hooks when BASS_TRACE is set;
    provide a degradable stub if the image's antenv lacks it."""
    import types

    try:
        import antenv.axon_hooks  # noqa: F401
        return
    except ImportError:
        pass
    try:
        import antenv
    except ImportError:
        return
    mod = types.ModuleType("antenv.axon_hooks")
    mod._hook = None
    mod.set_axon_ntff_profile_hook = lambda h: setattr(mod, "_hook", h)
    mod.get_axon_ntff_profile_hook = lambda: mod._hook
    sys.modules["antenv.axon_hooks"] = mod
    antenv.axon_hooks = mod


def kernel(x, edge_index, W1, b1, W2, b2):
    _ensure_axon_hooks_module()
    from concourse import bass_utils

    meta1, meta2, in_maps = _preprocess(x, edge_index, W1, b1, W2, b2)
    nc = _build(meta1, meta2, debug=False)
    res = bass_utils.run_bass_kernel_spmd(nc, in_maps, core_ids=list(range(NC)))
    out = np.concatenate([r["out"] for r in res.results], axis=0)
    return out.astype(np.float32)



# revision 33
# speedup vs baseline: 1.0199x; 1.0199x over previous
"""Trainium2 Bass kernel for a 2-layer GCN (PyG GCNConv x2 with self-loops).

Reference computation (N=100000 nodes, E=1600000 edges, f32):
    row, col = add_self_loops(edge_index)
    deg  = in-degree over col (incl. self loops); dis = rsqrt(deg)
    norm = dis[row] * dis[col]
    A_hat X = segment_sum(X[row] * norm, col)          # normalized aggregation
    h   = relu(A_hat X @ W1 + b1)                      # aggregate-then-transform
    out = (A_hat h) @ W2 + b2

Key algebraic identity: segment_sum((X W)[row]*norm, col) ==
segment_sum(X[row]*norm, col) @ W, so aggregation happens in the *input*
feature dim (64 / 128) which minimizes gather traffic.

Distribution over 8 NeuronCores: destination-node sharding (12500 nodes
per core).  Layer-1 aggregation / h / AllGather are pipelined per
node-quarter so the collectives hide behind layer-1 gather work; the
all-gathered h is copied from Shared DRAM to local DRAM before layer-2
gathers (Shared-space gather reads are ~2.7x slower than local ones).

Per-core aggregation (no scatter / no races):
  - edges (incl. self-loops) are bucketed host-side by (source-chunk k,
    dest-window w); windows are 128 destinations wide.
  - the device gathers source rows with gpsimd.dma_gather (G=1024 edges
    per call, 4 SWDGE queues round-robin, single_packet mode).
  - the symmetric norm dis[src]*dis[dst] is FACTORIZED out of the edge
    stream: dis[src] is pre-multiplied into the gathered rows (host-side
    for x, via an extra scalar-engine scale for h), dis[dst] is applied
    by the per-window output stages (activation per-partition scale +
    bias matmul against a 1/dis row instead of ones).  The selection
    tensor P[e, d] = (dest_e == d) is therefore a pure one-hot built
    with a SINGLE fp16 vector is_equal; each 128-edge chunk accumulates
    PSUM[feat, dest] += msg_chunk.T @ P on the tensor engine.
  - x is staged as fp16 row pairs xd[i] = [x~[i], x~[i+1]] so the
    256B-aligned gather element carries a 64-feature fp16 row (the pair
    tail is ballast) and every matmul runs at full fp16 PE rate.

All cores run one identical program (SPMD); per-(k,w) chunk counts are
equalized across cores host-side with padding edges whose dst=-1 makes
their one-hot rows all-zero.

Perf notes (HW-measured): the kernel is bound by the serialized
gpsimd dma_gather chain (~4us per 1024-edge call: ~2.5us Q7 descriptor
generation + non-overlapped SDMA drain/dispatch overhead).  Vector /
tensor / scalar engines all have slack.  Measured dead ends: per-slot
AP-scalar tensor_scalar (1.9us/op fixed), single_packet=0 (+13%),
larger descriptor rings (no change).
"""

import os
import sys

import numpy as np

for _p in ("/opt/trn_rl_repo", "/root/.axon_site/_ro/trn_rl_repo"):
    if os.path.isdir(_p) and _p not in sys.path:
        sys.path.insert(0, _p)

# ----------------------------------------------------------------------------
# Problem constants (hardcoded per the harness contract)
# ----------------------------------------------------------------------------
N = 100000          # nodes
NC = 8              # cores
NS = N // NC        # 12500 dest nodes per core
D0, D1, D2 = 64, 128, 256
WIN = 128           # dest window width
NW = (NS + WIN - 1) // WIN          # 98 windows per core
NSRC = 4            # source chunks (int16 gather index limit)
SC1 = N // NSRC     # 25000 rows per layer-1 source chunk
Q4 = NS // NSRC     # 3125 rows per all-gather chunk slice
G = int(os.environ.get("GCN_G", "1024"))   # edges per dma_gather call;
                    # hard ucode cap: calls >1024 descriptors deadlock on
                    # hardware even with a larger dynamic_dma_scratch_size
NQ = 4              # SWDGE queues (ucode max); gather calls round-robin
_SINGLE_PACKET = os.environ.get("GCN_SINGLE_PACKET", "1") == "1"
# prepare_only + trigger_dma measured SLOWER on hardware (the ring reclaim
# blocks the next prep through the previous transfer anyway, plus ~2x
# per-call overhead) and its completion-sem protocol mis-syncs; keep off.
_PREP = os.environ.get("GCN_PREP", "0") == "1"

# AllGather quarter boundaries in dest windows: AG q needs h rows
# [q*3125, (q+1)*3125) -> all windows < ceil((q+1)*3125/128) written.
WB = [0, 25, 49, 74, 98]
# Sub-groups bound the number of live PSUM window accumulators.  PSUM
# accumulation groups are zero-region (2KB bank) granular, so each open
# window needs its own bank: 5 window banks + 2 h-stage banks < 8.
SUBS = []
for _q in range(4):
    _a, _b = WB[_q], WB[_q + 1]
    for _s in range(_a, _b, 5):
        SUBS.append((_s, min(_s + 5, _b)))


# ----------------------------------------------------------------------------
# Host-side preprocessing: sharding + edge bucketing
# ----------------------------------------------------------------------------
def _plan_layer(src_all, dloc_all, core_all, src_chunk_of, src_idx_of):
    """Bucket each core's edges by (source-chunk k, dest-window w), pad each
    bucket to a multiple of 128 edges AND to identical chunk counts across
    all cores (SPMD requires one program).

    The symmetric norm dis[src]*dis[dst] is factorized out of the edge
    stream entirely: dis[src] is folded into the gathered source rows
    (x / h pre-scaled), dis[dst] into the per-window output stages.  Each
    edge therefore only carries (idx, dst); P is a pure one-hot.

    Returns (meta, per_core_arrays):
      meta = {"Tk": [...], "segs": [[(w, n_chunks), ...] per k]}
      per_core_arrays[c] = {"idx": [...], "dst": [...]}
    """
    NWl = NW
    counts = np.zeros((NC, NSRC, NWl), dtype=np.int64)
    per_core = []
    for c in range(NC):
        sel = core_all == c
        src = src_all[sel]
        dloc = dloc_all[sel]
        k = src_chunk_of(src)
        w = dloc // WIN
        idxl = src_idx_of(src)
        order = np.lexsort((w, k))
        k, w, idxl, dloc = k[order], w[order], idxl[order], dloc[order]
        key = k * NWl + w
        counts[c] = np.bincount(key, minlength=NSRC * NWl).reshape(NSRC, NWl)
        per_core.append((k, w, idxl, dloc, key))

    nch = (counts.max(axis=0) + 127) // 128          # [NSRC, NW] chunks per bucket
    Tk = (nch.sum(axis=1) * 128).astype(np.int64)    # padded edges per chunk k
    segs = [[(int(w), int(nch[k, w])) for w in range(NWl) if nch[k, w] > 0]
            for k in range(NSRC)]

    base = np.zeros((NSRC, NWl), dtype=np.int64)
    for k in range(NSRC):
        base[k] = np.concatenate(([0], np.cumsum(nch[k] * 128)[:-1]))

    out = []
    for c in range(NC):
        k, w, idxl, dloc, key = per_core[c]
        cnt = counts[c].reshape(-1)
        starts = np.concatenate(([0], np.cumsum(cnt)[:-1]))
        pos_in_bucket = np.arange(len(key)) - starts[key]
        tgt = base.reshape(-1)[key] + pos_in_bucket   # position inside k-stream
        arrs = {"idx": [], "dst": []}
        for kk in range(NSRC):
            T = int(Tk[kk])
            idx16 = np.zeros(T, dtype=np.int16)
            dwf = np.full(T, -1.0, dtype=np.float32)
            m = k == kk
            t = tgt[m]
            idx16[t] = idxl[m].astype(np.int16)
            dwf[t] = (dloc[m] - (w[m] * WIN)).astype(np.float32)
            # device layouts: idx wraps by 16 (replicated to 128 partitions
            # for the 8 gpsimd cores), dst wraps by 128
            arrs["idx"].append(np.ascontiguousarray(
                np.tile(idx16.reshape(T // 16, 16).T, (8, 1))))
            arrs["dst"].append(np.ascontiguousarray(
                dwf.reshape(T // 128, 128).T.astype(np.float16)))
        out.append(arrs)
    return {"Tk": [int(t) for t in Tk], "segs": segs}, out


def _preprocess(x, edge_index, W1, b1, W2, b2):
    row = np.asarray(edge_index[0], dtype=np.int64)
    col = np.asarray(edge_index[1], dtype=np.int64)
    deg = (np.bincount(col, minlength=N) + 1).astype(np.float32)  # + self loop
    dis = (1.0 / np.sqrt(deg)).astype(np.float32)

    loop = np.arange(N, dtype=np.int64)
    rows = np.concatenate([row, loop])
    cols = np.concatenate([col, loop])
    core = (cols // NS).astype(np.int64)
    dloc = cols - core * NS

    # L1 source chunks are INTERLEAVED (src % NSRC) rather than contiguous:
    # a core's self-loop sources all fall in one contiguous chunk, which
    # would skew the cross-core bucket maxima and inflate SPMD padding.
    meta1, arrs1 = _plan_layer(
        rows, dloc, core,
        src_chunk_of=lambda s: s % NSRC,
        src_idx_of=lambda s: s // NSRC,
    )
    meta2, arrs2 = _plan_layer(
        rows, dloc, core,
        src_chunk_of=lambda s: (s % NS) // Q4,
        src_idx_of=lambda s: (s // NS) * Q4 + (s % NS) % Q4,
    )

    x = np.asarray(x, dtype=np.float32)
    # fp16 pair-duplicated, dis[src]-prescaled x: xd[i] = [x~[i], x~[i+1]];
    # the gather element is 128 fp16 = 256 bytes, of which the first 64 are
    # the row we want.
    xf16 = (x * dis[:, None]).astype(np.float16)
    xd = np.zeros((N, 2 * D0), dtype=np.float16)
    xd[:, :D0] = xf16
    xd[:-1, D0:] = xf16[1:]

    shared = {
        "xd": np.ascontiguousarray(xd),
        "W1": np.ascontiguousarray(np.asarray(W1, dtype=np.float32)),
        "b1": np.ascontiguousarray(
            np.asarray(b1, dtype=np.float32).reshape(1, D1)),
        "W2": np.ascontiguousarray(np.asarray(W2, dtype=np.float32)),
        "b2": np.ascontiguousarray(
            np.asarray(b2, dtype=np.float32).reshape(1, D2)),
    }
    in_maps = []
    for c in range(NC):
        m = dict(shared)
        # Per-core dis layouts for the dst-side factor (shard padded to
        # NW*WIN = 12544 rows):
        #   dis_act[p, w] = dis[c*NS + w*128 + p]  (activation per-part scale)
        #   disinv[0, j]  = 1/dis[c*NS + j]        (bias-matmul stationary row)
        dc = np.ones(NW * WIN, dtype=np.float32)
        dc[:NS] = dis[c * NS:(c + 1) * NS]
        di = np.zeros(NW * WIN, dtype=np.float32)
        di[:NS] = 1.0 / dis[c * NS:(c + 1) * NS]
        m["dis_act"] = np.ascontiguousarray(dc.reshape(NW, WIN).T)
        m["disinv"] = np.ascontiguousarray(di.reshape(1, NW * WIN))
        for kk in range(NSRC):
            m[f"idx1_{kk}"] = arrs1[c]["idx"][kk]
            m[f"dst1_{kk}"] = arrs1[c]["dst"][kk]
            m[f"idx2_{kk}"] = arrs2[c]["idx"][kk]
            m[f"dst2_{kk}"] = arrs2[c]["dst"][kk]
        in_maps.append(m)
    return meta1, meta2, in_maps


# ----------------------------------------------------------------------------
# Device program
# ----------------------------------------------------------------------------
def _build(meta1, meta2, debug=False, stage="full"):
    from contextlib import ExitStack

    import concourse.bacc as bacc
    import concourse.bass as bass
    import concourse.mybir as mybir
    import concourse.tile as tile

    f32, f16, i16 = mybir.dt.float32, mybir.dt.float16, mybir.dt.int16
    GC = G // 128

    nc = bacc.Bacc("TRN2", target_bir_lowering=False, debug=debug,
                   num_devices=NC, num_swdge_queues=NQ,
                   dynamic_dma_scratch_size=int(
                       os.environ.get("GCN_SCRATCH", str(16 * G))))

    xd_d = nc.dram_tensor("xd", [N, 2 * D0], f16, kind="ExternalInput")
    w1_d = nc.dram_tensor("W1", [D0, D1], f32, kind="ExternalInput")
    b1_d = nc.dram_tensor("b1", [1, D1], f32, kind="ExternalInput")
    w2_d = nc.dram_tensor("W2", [D1, D2], f32, kind="ExternalInput")
    b2_d = nc.dram_tensor("b2", [1, D2], f32, kind="ExternalInput")
    dis_act_d = nc.dram_tensor("dis_act", [WIN, NW], f32, kind="ExternalInput")
    disinv_d = nc.dram_tensor("disinv", [1, NW * WIN], f32, kind="ExternalInput")

    idx1_d, dst1_d, idx2_d, dst2_d = [], [], [], []
    for k in range(NSRC):
        T1, T2 = meta1["Tk"][k], meta2["Tk"][k]
        idx1_d.append(nc.dram_tensor(f"idx1_{k}", [128, T1 // 16], i16, kind="ExternalInput"))
        dst1_d.append(nc.dram_tensor(f"dst1_{k}", [128, T1 // 128], f16, kind="ExternalInput"))
        idx2_d.append(nc.dram_tensor(f"idx2_{k}", [128, T2 // 16], i16, kind="ExternalInput"))
        dst2_d.append(nc.dram_tensor(f"dst2_{k}", [128, T2 // 128], f16, kind="ExternalInput"))

    h_own = nc.dram_tensor("h_own", [NS, D1], f16, kind="Internal")
    hf = [nc.dram_tensor(f"hf{q}", [NC * Q4, D1], f16, kind="Internal",
                         addr_space="Shared") for q in range(NSRC)]
    hl = [nc.dram_tensor(f"hl{q}", [NC * Q4, D1], f16, kind="Internal")
          for q in range(NSRC)]
    if stage == "A":
        out_d = nc.dram_tensor("out", [D0, NW * WIN], f32, kind="ExternalOutput")
    elif stage == "AB":
        out_d = nc.dram_tensor("out", [NS, D1], f16, kind="ExternalOutput")
    elif stage == "ABC":
        out_d = nc.dram_tensor("out", [NC * Q4, D1], f16, kind="ExternalOutput")
    elif stage == "AD":
        out_d = nc.dram_tensor("out", [D1, NW * WIN], f32, kind="ExternalOutput")
    else:
        out_d = nc.dram_tensor("out", [NS, D2], f32, kind="ExternalOutput")
    acc2_dump = (nc.dram_tensor("acc2dump", [D1, NW * WIN], f32,
                                kind="ExternalOutput")
                 if stage == "full+dump" else None)
    if stage == "full+dump":
        stage = "full"

    # per-bucket chunk counts / prefix offsets per k-stream
    nch1 = np.zeros((NSRC, NW), dtype=np.int64)
    for k in range(NSRC):
        for (w, n) in meta1["segs"][k]:
            nch1[k][w] = n
    pre1 = np.zeros((NSRC, NW + 1), dtype=np.int64)
    for k in range(NSRC):
        pre1[k][1:] = np.cumsum(nch1[k])
    # first/last source-chunk contributing to each window (PSUM start/stop)
    fk = [min(k for k in range(NSRC) if nch1[k][w] > 0) for w in range(NW)]
    lk = [max(k for k in range(NSRC) if nch1[k][w] > 0) for w in range(NW)]

    qsems = [nc.alloc_semaphore(f"gq{i}") for i in range(NQ)]
    qstate = {"n": 0, "cum": [0] * NQ}
    drain_hl_hook = [lambda: None]

    def gather_call(gt_ap, src_ap, idx_ap, mlen, elem):
        """Issue one gather call.  With _PREP, descriptor generation is
        decoupled from the transfer (prepare_only + trigger) so the Pool
        engine never blocks on the DMA; the data-completion sync is the
        caller's job: we emit an explicit PE wait_ge on the queue's DMA
        semaphore right before the first consuming matmul (Tile's
        lane-sem waits are pre-bumped away by the framework)."""
        q = qstate["n"] % NQ
        qstate["n"] += 1
        drain_hl_hook[0]()
        if _PREP:
            nc.gpsimd.dma_gather(
                gt_ap, src_ap, idx_ap, mlen, mlen, elem,
                elem_step=src_ap.ap[0][0], queue_num=q,
                single_packet=_SINGLE_PACKET,
                prepare_only=True, sem=qsems[q])
            nc.gpsimd.trigger_dma(count=None, queue_num=q)
            qstate["cum"][q] += 16
            nc.tensor.wait_ge(qsems[q], qstate["cum"][q])
        else:
            nc.gpsimd.dma_gather(
                gt_ap, src_ap, idx_ap, mlen, mlen, elem,
                elem_step=src_ap.ap[0][0], queue_num=q,
                single_packet=_SINGLE_PACKET)

    with tile.TileContext(nc) as tc:
        with ExitStack() as top:
            const = top.enter_context(tc.tile_pool(name="const", bufs=1))
            w1_t = const.tile([D0, D1], f32)
            nc.sync.dma_start(w1_t[:], w1_d[:])
            b1_t = const.tile([1, D1], f32)
            nc.sync.dma_start(b1_t[:], b1_d[:])
            w2_t = const.tile([D1, D2], f32)
            nc.sync.dma_start(w2_t[:], w2_d[:])
            b2_t = const.tile([1, D2], f32)
            nc.sync.dma_start(b2_t[:], b2_d[:])
            dis_act_t = const.tile([WIN, NW], f32)
            nc.sync.dma_start(dis_act_t[:], dis_act_d[:])
            disinv_t = const.tile([1, NW * WIN], f32)
            nc.sync.dma_start(disinv_t[:], disinv_d[:])
            iota_t = const.tile([128, GC, WIN], f16)
            nc.gpsimd.iota(iota_t[:], pattern=[[0, GC], [1, WIN]], base=0,
                           channel_multiplier=0,
                           allow_small_or_imprecise_dtypes=True)

            accp = top.enter_context(tc.tile_pool(name="acc", bufs=1))
            acc1 = accp.tile([D0, NW * WIN], f32)
            nc.vector.memset(acc1[:], 0.0)
            acc2 = accp.tile([D1, NW * WIN], f32)
            nc.vector.memset(acc2[:], 0.0)

            xb = xd_d[:]
            x_aps = [bass.AP(xb.tensor, k * 2 * D0,
                             [[NSRC * 2 * D0, SC1], [1, 2 * D0]])
                     for k in range(NSRC)]

            def bcast(col_slice, mc):
                return bass.AP(col_slice.tensor, col_slice.offset,
                               [list(col_slice.ap[0]), [1, mc], [0, WIN]])

            # Deferred AllGather emission: the collective trigger sits in
            # the Pool queue and head-of-line blocks subsequent gather
            # calls while waiting on the h DMAs; emitting it AFTER the
            # next phase's first gather call hides that wait.
            pending_ag = []
            pending_hl = []
            HL_PIECES = 1

            def flush_ag():
                while pending_ag:
                    qq = pending_ag.pop(0)
                    nc.gpsimd.collective_compute(
                        "AllGather", mybir.AluOpType.bypass,
                        replica_groups=[list(range(NC))],
                        ins=[h_own[qq * Q4:(qq + 1) * Q4, :]],
                        outs=[hf[qq][:, :]],
                    )
                    for pc in range(HL_PIECES):
                        pending_hl.append((qq, pc))
                    while pending_hl:
                        drain_hl_hook[0]()

            # The 6.4MB hf->hl staging copies monopolize the SDMA engines
            # when issued as one DMA (the big post-AllGather gather stalls in
            # the profile).  Instead they are chunked and trickled out, one
            # piece per subsequent gather call, so gather packets interleave.
            def drain_hl():
                if pending_hl:
                    qq, pc = pending_hl.pop(0)
                    r0 = pc * (NC * Q4 // HL_PIECES)
                    r1 = r0 + NC * Q4 // HL_PIECES
                    nc.sync.dma_start(hl[qq][r0:r1, :], hf[qq][r0:r1, :])

            drain_hl_hook[0] = drain_hl

            # ---------------- Layer 1 + h + AllGather, per quarter --------
            with ExitStack() as l1s:
                mp = l1s.enter_context(tc.tile_pool(name="meta1", bufs=6))
                gp = l1s.enter_context(tc.tile_pool(name="g1", bufs=8))
                pp = l1s.enter_context(tc.tile_pool(name="p1", bufs=6))
                psp1 = l1s.enter_context(
                    tc.tile_pool(name="ps1", bufs=6, space="PSUM"))
                psb = l1s.enter_context(
                    tc.tile_pool(name="psb", bufs=2, space="PSUM"))
                hp = l1s.enter_context(tc.tile_pool(name="hb", bufs=4))

                for qi in range(4):
                    g0, g1 = WB[qi], WB[qi + 1]
                    for k in range(NSRC):
                        c0, c1 = int(pre1[k][g0]), int(pre1[k][g1])
                        total = c1 - c0
                        if total == 0:
                            continue
                        idx_t = mp.tile([128, total * 8], i16, tag="idx")
                        dst_t = mp.tile([128, total], f16, tag="dst")
                        NPC = 4
                        cw = (total + NPC - 1) // NPC
                        for pc in range(NPC):
                            a, b = pc * cw, min((pc + 1) * cw, total)
                            if a >= b:
                                break
                            nc.sync.dma_start(
                                idx_t[:, a * 8:b * 8],
                                idx1_d[k][:, (c0 + a) * 8:(c0 + b) * 8])
                            nc.sync.dma_start(dst_t[:, a:b],
                                              dst1_d[k][:, c0 + a:c0 + b])
                        jj = 0
                        gt = None
                        P8 = None
                        for w in range(g0, g1):
                            nchk = int(nch1[k][w])
                            if nchk == 0:
                                continue
                            ps = psp1.tile([D0, WIN], f32, tag="ps1")
                            for j in range(nchk):
                                t, slot = divmod(jj, GC)
                                if slot == 0:
                                    mc = min(GC, total - t * GC)
                                    gt = gp.tile([128, GC, 2 * D0], f16,
                                                 tag="gt")
                                    gather_call(
                                        gt[:, :mc, :], x_aps[k],
                                        idx_t[:, t * GC * 8:
                                              t * GC * 8 + mc * 8],
                                        mc * 128, 2 * D0)
                                    flush_ag()
                                    P8 = pp.tile([128, GC, WIN], f16,
                                                 tag="P")
                                    nc.vector.tensor_tensor(
                                        P8[:, :mc, :], iota_t[:, :mc, :],
                                        bcast(dst_t[:, t * GC:t * GC + mc], mc),
                                        mybir.AluOpType.is_equal)
                                nc.tensor.matmul(
                                    ps[:], gt[:, slot, 0:D0],
                                    P8[:, slot, :],
                                    start=(j == 0), stop=(j == nchk - 1))
                                jj += 1
                            nc.vector.tensor_tensor(
                                acc1[:, w * WIN:(w + 1) * WIN],
                                acc1[:, w * WIN:(w + 1) * WIN], ps[:],
                                mybir.AluOpType.add)

                    # ---- h~ = dis * relu(dis*(acc1.T @ W1) + b1) ----
                    # ps = acc1.T@W1 + (1/dis)*b1; relu(ps*dis) applies the
                    # dst-side dis INSIDE the relu, the extra mul applies the
                    # src-side dis for layer-2 gathers.
                    for w in range(g0, g1):
                        M = min(WIN, NS - w * WIN)
                        ps = psb.tile([M, D1], f32, tag="psb")
                        nc.tensor.matmul(ps[:], acc1[:, w * WIN:w * WIN + M],
                                         w1_t[:], start=True, stop=False)
                        nc.tensor.matmul(
                            ps[:], disinv_t[:, w * WIN:w * WIN + M], b1_t[:],
                            start=False, stop=True)
                        ht = hp.tile([M, D1], f16, tag="ht")
                        nc.scalar.activation(ht[:], ps[:],
                                             mybir.ActivationFunctionType.Relu,
                                             scale=dis_act_t[0:M, w:w + 1])
                        ht2 = hp.tile([M, D1], f16, tag="ht2")
                        nc.scalar.mul(ht2[:], ht[:],
                                      dis_act_t[0:M, w:w + 1])
                        nc.sync.dma_start(h_own[w * WIN:w * WIN + M, :], ht2[:])

                    pending_ag.append(qi)

            if stage in ("A", "AB", "ABC"):
                flush_ag()
                while pending_hl:
                    drain_hl()
            if stage == "A":
                nc.sync.dma_start(out_d[:], acc1[:])
            elif stage == "AB":
                nc.sync.dma_start(out_d[:], h_own[:])
            elif stage == "ABC":
                nc.sync.dma_start(out_d[:], hl[0][:])

            # ------- Layer 2 (+ folded out-stage on the last k pass) -------
            with ExitStack() as l2s:
                if stage in ("A", "AB", "ABC"):
                    meta2 = {"Tk": [0] * NSRC, "segs": [[] for _ in range(NSRC)]}
                nch2 = np.zeros((NSRC, NW), dtype=np.int64)
                for k in range(NSRC):
                    for (w, n) in meta2["segs"][k]:
                        nch2[k][w] = n
                mp2 = l2s.enter_context(tc.tile_pool(name="meta2", bufs=2))
                gp2 = l2s.enter_context(tc.tile_pool(name="g2", bufs=8))
                pp2 = l2s.enter_context(tc.tile_pool(name="p2", bufs=6))
                psp = l2s.enter_context(
                    tc.tile_pool(name="ps2", bufs=6, space="PSUM"))
                op = l2s.enter_context(tc.tile_pool(name="ob", bufs=4))
                pso = l2s.enter_context(
                    tc.tile_pool(name="pso", bufs=2, space="PSUM"))

                for k in range(NSRC):
                    Tk = meta2["Tk"][k]
                    last_k = k == NSRC - 1
                    if Tk > 0:
                        total = Tk // 128
                        idx_t = mp2.tile([128, Tk // 16], i16, tag="idx2")
                        dst_t = mp2.tile([128, total], f16, tag="dst2")
                        NPC = 8
                        cw = (total + NPC - 1) // NPC
                        for pc in range(NPC):
                            a, b = pc * cw, min((pc + 1) * cw, total)
                            if a >= b:
                                break
                            nc.sync.dma_start(idx_t[:, a * 8:b * 8],
                                              idx2_d[k][:, a * 8:b * 8])
                            nc.sync.dma_start(dst_t[:, a:b],
                                              dst2_d[k][:, a:b])
                        hl_ap = hl[k][:]
                        jj = 0
                        gt = None
                        P8 = None
                    for w in range(NW):
                        nchk = int(nch2[k][w]) if Tk > 0 else 0
                        if nchk > 0:
                            ps = psp.tile([D1, WIN], f32, tag="ps2")
                            for j in range(nchk):
                                t, slot = divmod(jj, GC)
                                if slot == 0:
                                    mc = min(GC, total - t * GC)
                                    gt = gp2.tile([128, GC, D1], f16,
                                                  tag="gt2")
                                    gather_call(
                                        gt[:, :mc, :], hl_ap,
                                        idx_t[:, t * GC * 8:t * GC * 8 + mc * 8],
                                        mc * 128, D1)
                                    flush_ag()
                                    P8 = pp2.tile([128, GC, WIN], f16,
                                                  tag="P2")
                                    nc.vector.tensor_tensor(
                                        P8[:, :mc, :], iota_t[:, :mc, :],
                                        bcast(dst_t[:, t * GC:t * GC + mc], mc),
                                        mybir.AluOpType.is_equal)
                                nc.tensor.matmul(ps[:], gt[:, slot, :],
                                                 P8[:, slot, :],
                                                 start=(j == 0),
                                                 stop=(j == nchk - 1))
                                jj += 1
                            nc.vector.tensor_tensor(
                                acc2[:, w * WIN:(w + 1) * WIN],
                                acc2[:, w * WIN:(w + 1) * WIN], ps[:],
                                mybir.AluOpType.add)
                        if last_k and stage == "full":
                            # acc2[:, w] is final: out = dis*(acc2.T@W2) + b2
                            # emitted as dis*(acc2.T@W2 + (1/dis)*b2).
                            M = min(WIN, NS - w * WIN)
                            pso_t = pso.tile([M, D2], f32, tag="pso")
                            nc.tensor.matmul(
                                pso_t[:], acc2[:, w * WIN:w * WIN + M],
                                w2_t[:], start=True, stop=False)
                            nc.tensor.matmul(
                                pso_t[:], disinv_t[:, w * WIN:w * WIN + M],
                                b2_t[:], start=False, stop=True)
                            ot = op.tile([M, D2], f32, tag="ot")
                            nc.scalar.mul(ot[:], pso_t[:],
                                          dis_act_t[0:M, w:w + 1])
                            nc.sync.dma_start(
                                out_d[w * WIN:w * WIN + M, :], ot[:])

            if stage == "AD":
                nc.sync.dma_start(out_d[:], acc2[:])
            if acc2_dump is not None:
                nc.sync.dma_start(acc2_dump[:], acc2[:])

    nc.compile()
    return nc


# ----------------------------------------------------------------------------
# Entry point
# ----------------------------------------------------------------------------
def _ensure_axon_hooks_module():
    """bass_utils hard-imports antenv.axon_hooks when BASS_TRACE is set;
    provide a degradable stub if the image's antenv lacks it."""
    import types

    try:
        import antenv.axon_hooks  # noqa: F401
        return
    except ImportError:
        pass
    try:
        import antenv
    except ImportError:
        return
    mod = types.ModuleType("antenv.axon_hooks")
    mod._hook = None
    mod.set_axon_ntff_profile_hook = lambda h: setattr(mod, "_hook", h)
    mod.get_axon_ntff_profile_hook = lambda: mod._hook
    sys.modules["antenv.axon_hooks"] = mod
    antenv.axon_hooks = mod


def kernel(x, edge_index, W1, b1, W2, b2):
    _ensure_axon_hooks_module()
    from concourse import bass_utils

    meta1, meta2, in_maps = _preprocess(x, edge_index, W1, b1, W2, b2)
    nc = _build(meta1, meta2, debug=False)
    res = bass_utils.run_bass_kernel_spmd(nc, in_maps, core_ids=list(range(NC)))
    out = np.concatenate([r["out"] for r in res.results], axis=0)
    return out.astype(np.float32)



# revision 36
# speedup vs baseline: 1.0305x; 1.0103x over previous
"""Trainium2 Bass kernel for a 2-layer GCN (PyG GCNConv x2 with self-loops).

Reference computation (N=100000 nodes, E=1600000 edges, f32):
    row, col = add_self_loops(edge_index)
    deg  = in-degree over col (incl. self loops); dis = rsqrt(deg)
    norm = dis[row] * dis[col]
    A_hat X = segment_sum(X[row] * norm, col)          # normalized aggregation
    h   = relu(A_hat X @ W1 + b1)                      # aggregate-then-transform
    out = (A_hat h) @ W2 + b2

Key algebraic identity: segment_sum((X W)[row]*norm, col) ==
segment_sum(X[row]*norm, col) @ W, so aggregation happens in the *input*
feature dim (64 / 128) which minimizes gather traffic.

Distribution over 8 NeuronCores: destination-node sharding (12500 nodes
per core).  Layer-1 aggregation / h / AllGather are pipelined per
node-quarter so the collectives hide behind layer-1 gather work; the
all-gathered h is copied from Shared DRAM to local DRAM before layer-2
gathers (Shared-space gather reads are ~2.7x slower than local ones).

Per-core aggregation (no scatter / no races):
  - edges (incl. self-loops) are bucketed host-side by (source-chunk k,
    dest-window w); windows are 128 destinations wide.
  - the device gathers source rows with gpsimd.dma_gather (G=1024 edges
    per call, 4 SWDGE queues round-robin, single_packet mode).
  - the symmetric norm dis[src]*dis[dst] is FACTORIZED out of the edge
    stream: dis[src] is pre-multiplied into the gathered rows (host-side
    for x, via an extra scalar-engine scale for h), dis[dst] is applied
    by the per-window output stages (activation per-partition scale +
    bias matmul against a 1/dis row instead of ones).  The selection
    tensor P[e, d] = (dest_e == d) is therefore a pure one-hot built
    with a SINGLE fp16 vector is_equal; each 128-edge chunk accumulates
    PSUM[feat, dest] += msg_chunk.T @ P on the tensor engine.
  - x is staged as fp16 row pairs xd[i] = [x~[i], x~[i+1]] so the
    256B-aligned gather element carries a 64-feature fp16 row (the pair
    tail is ballast) and every matmul runs at full fp16 PE rate.

All cores run one identical program (SPMD); per-(k,w) chunk counts are
equalized across cores host-side with padding edges whose dst=-1 makes
their one-hot rows all-zero.

Perf notes (HW-measured): the kernel is bound by the gpsimd dma_gather
chain.  A standalone microbench shows the floor is ~2.17us per
1024-edge call (per-queue ring cycle 8.66us = ~2.5us Q7 descriptor
generation + ~6us SDMA drain, overlapped 4x across the SWDGE queues;
generation DOES overlap across the per-queue Q7 core pairs).  The
kernel's steady-state bursts already run at that floor; the ~3.9us
mean cadence comes from ~0.9ms of stalls clustered around the
AllGather / hf->hl copy windows and segment starts.  Vector / tensor /
scalar engines all have slack.  Measured dead ends: per-slot AP-scalar
tensor_scalar (1.9us/op fixed cost), single_packet=0 (+13%), larger
descriptor rings (no change), chunking the hl copies into trickled
0.8MB pieces (+4.7%), chunking the idx/dst meta loads (+2.6%) -- more,
smaller DMAs worsen scheduler/lane-semaphore coupling.
"""

import os
import sys

import numpy as np

for _p in ("/opt/trn_rl_repo", "/root/.axon_site/_ro/trn_rl_repo"):
    if os.path.isdir(_p) and _p not in sys.path:
        sys.path.insert(0, _p)

# ----------------------------------------------------------------------------
# Problem constants (hardcoded per the harness contract)
# ----------------------------------------------------------------------------
N = 100000          # nodes
NC = 8              # cores
NS = N // NC        # 12500 dest nodes per core
D0, D1, D2 = 64, 128, 256
WIN = 128           # dest window width
NW = (NS + WIN - 1) // WIN          # 98 windows per core
NSRC = 4            # source chunks (int16 gather index limit)
SC1 = N // NSRC     # 25000 rows per layer-1 source chunk
Q4 = NS // NSRC     # 3125 rows per all-gather chunk slice
G = int(os.environ.get("GCN_G", "1024"))   # edges per dma_gather call;
                    # hard ucode cap: calls >1024 descriptors deadlock on
                    # hardware even with a larger dynamic_dma_scratch_size
NQ = 4              # SWDGE queues (ucode max); gather calls round-robin
_SINGLE_PACKET = os.environ.get("GCN_SINGLE_PACKET", "1") == "1"
# prepare_only + trigger_dma measured SLOWER on hardware (the ring reclaim
# blocks the next prep through the previous transfer anyway, plus ~2x
# per-call overhead) and its completion-sem protocol mis-syncs; keep off.
_PREP = os.environ.get("GCN_PREP", "0") == "1"

# AllGather quarter boundaries in dest windows: AG q needs h rows
# [q*3125, (q+1)*3125) -> all windows < ceil((q+1)*3125/128) written.
WB = [0, 25, 49, 74, 98]
# Sub-groups bound the number of live PSUM window accumulators.  PSUM
# accumulation groups are zero-region (2KB bank) granular, so each open
# window needs its own bank: 5 window banks + 2 h-stage banks < 8.
SUBS = []
for _q in range(4):
    _a, _b = WB[_q], WB[_q + 1]
    for _s in range(_a, _b, 5):
        SUBS.append((_s, min(_s + 5, _b)))


# ----------------------------------------------------------------------------
# Host-side preprocessing: sharding + edge bucketing
# ----------------------------------------------------------------------------
def _plan_layer(src_all, dloc_all, core_all, src_chunk_of, src_idx_of):
    """Bucket each core's edges by (source-chunk k, dest-window w), pad each
    bucket to a multiple of 128 edges AND to identical chunk counts across
    all cores (SPMD requires one program).

    The symmetric norm dis[src]*dis[dst] is factorized out of the edge
    stream entirely: dis[src] is folded into the gathered source rows
    (x / h pre-scaled), dis[dst] into the per-window output stages.  Each
    edge therefore only carries (idx, dst); P is a pure one-hot.

    Returns (meta, per_core_arrays):
      meta = {"Tk": [...], "segs": [[(w, n_chunks), ...] per k]}
      per_core_arrays[c] = {"idx": [...], "dst": [...]}
    """
    NWl = NW
    counts = np.zeros((NC, NSRC, NWl), dtype=np.int64)
    per_core = []
    for c in range(NC):
        sel = core_all == c
        src = src_all[sel]
        dloc = dloc_all[sel]
        k = src_chunk_of(src)
        w = dloc // WIN
        idxl = src_idx_of(src)
        order = np.lexsort((w, k))
        k, w, idxl, dloc = k[order], w[order], idxl[order], dloc[order]
        key = k * NWl + w
        counts[c] = np.bincount(key, minlength=NSRC * NWl).reshape(NSRC, NWl)
        per_core.append((k, w, idxl, dloc, key))

    nch = (counts.max(axis=0) + 127) // 128          # [NSRC, NW] chunks per bucket
    Tk = (nch.sum(axis=1) * 128).astype(np.int64)    # padded edges per chunk k
    segs = [[(int(w), int(nch[k, w])) for w in range(NWl) if nch[k, w] > 0]
            for k in range(NSRC)]

    base = np.zeros((NSRC, NWl), dtype=np.int64)
    for k in range(NSRC):
        base[k] = np.concatenate(([0], np.cumsum(nch[k] * 128)[:-1]))

    out = []
    for c in range(NC):
        k, w, idxl, dloc, key = per_core[c]
        cnt = counts[c].reshape(-1)
        starts = np.concatenate(([0], np.cumsum(cnt)[:-1]))
        pos_in_bucket = np.arange(len(key)) - starts[key]
        tgt = base.reshape(-1)[key] + pos_in_bucket   # position inside k-stream
        arrs = {"idx": [], "dst": []}
        for kk in range(NSRC):
            T = int(Tk[kk])
            idx16 = np.zeros(T, dtype=np.int16)
            dwf = np.full(T, -1.0, dtype=np.float32)
            m = k == kk
            t = tgt[m]
            idx16[t] = idxl[m].astype(np.int16)
            dwf[t] = (dloc[m] - (w[m] * WIN)).astype(np.float32)
            # device layouts: idx wraps by 16 (replicated to 128 partitions
            # for the 8 gpsimd cores), dst wraps by 128
            arrs["idx"].append(np.ascontiguousarray(
                np.tile(idx16.reshape(T // 16, 16).T, (8, 1))))
            arrs["dst"].append(np.ascontiguousarray(
                dwf.reshape(T // 128, 128).T.astype(np.float16)))
        out.append(arrs)
    return {"Tk": [int(t) for t in Tk], "segs": segs}, out


def _preprocess(x, edge_index, W1, b1, W2, b2):
    row = np.asarray(edge_index[0], dtype=np.int64)
    col = np.asarray(edge_index[1], dtype=np.int64)
    deg = (np.bincount(col, minlength=N) + 1).astype(np.float32)  # + self loop
    dis = (1.0 / np.sqrt(deg)).astype(np.float32)

    loop = np.arange(N, dtype=np.int64)
    rows = np.concatenate([row, loop])
    cols = np.concatenate([col, loop])
    core = (cols // NS).astype(np.int64)
    dloc = cols - core * NS

    # L1 source chunks are INTERLEAVED (src % NSRC) rather than contiguous:
    # a core's self-loop sources all fall in one contiguous chunk, which
    # would skew the cross-core bucket maxima and inflate SPMD padding.
    meta1, arrs1 = _plan_layer(
        rows, dloc, core,
        src_chunk_of=lambda s: s % NSRC,
        src_idx_of=lambda s: s // NSRC,
    )
    meta2, arrs2 = _plan_layer(
        rows, dloc, core,
        src_chunk_of=lambda s: (s % NS) // Q4,
        src_idx_of=lambda s: (s // NS) * Q4 + (s % NS) % Q4,
    )

    x = np.asarray(x, dtype=np.float32)
    # fp16 pair-duplicated, dis[src]-prescaled x: xd[i] = [x~[i], x~[i+1]];
    # the gather element is 128 fp16 = 256 bytes, of which the first 64 are
    # the row we want.
    xf16 = (x * dis[:, None]).astype(np.float16)
    xd = np.zeros((N, 2 * D0), dtype=np.float16)
    xd[:, :D0] = xf16
    xd[:-1, D0:] = xf16[1:]

    shared = {
        "xd": np.ascontiguousarray(xd),
        "W1": np.ascontiguousarray(np.asarray(W1, dtype=np.float32)),
        "b1": np.ascontiguousarray(
            np.asarray(b1, dtype=np.float32).reshape(1, D1)),
        "W2": np.ascontiguousarray(np.asarray(W2, dtype=np.float32)),
        "b2": np.ascontiguousarray(
            np.asarray(b2, dtype=np.float32).reshape(1, D2)),
    }
    in_maps = []
    for c in range(NC):
        m = dict(shared)
        # Per-core dis layouts for the dst-side factor (shard padded to
        # NW*WIN = 12544 rows):
        #   dis_act[p, w] = dis[c*NS + w*128 + p]  (activation per-part scale)
        #   disinv[0, j]  = 1/dis[c*NS + j]        (bias-matmul stationary row)
        dc = np.ones(NW * WIN, dtype=np.float32)
        dc[:NS] = dis[c * NS:(c + 1) * NS]
        di = np.zeros(NW * WIN, dtype=np.float32)
        di[:NS] = 1.0 / dis[c * NS:(c + 1) * NS]
        m["dis_act"] = np.ascontiguousarray(dc.reshape(NW, WIN).T)
        m["disinv"] = np.ascontiguousarray(di.reshape(1, NW * WIN))
        for kk in range(NSRC):
            m[f"idx1_{kk}"] = arrs1[c]["idx"][kk]
            m[f"dst1_{kk}"] = arrs1[c]["dst"][kk]
            m[f"idx2_{kk}"] = arrs2[c]["idx"][kk]
            m[f"dst2_{kk}"] = arrs2[c]["dst"][kk]
        in_maps.append(m)
    return meta1, meta2, in_maps


# ----------------------------------------------------------------------------
# Device program
# ----------------------------------------------------------------------------
def _build(meta1, meta2, debug=False, stage="full"):
    from contextlib import ExitStack

    import concourse.bacc as bacc
    import concourse.bass as bass
    import concourse.mybir as mybir
    import concourse.tile as tile

    f32, f16, i16 = mybir.dt.float32, mybir.dt.float16, mybir.dt.int16
    GC = G // 128

    nc = bacc.Bacc("TRN2", target_bir_lowering=False, debug=debug,
                   num_devices=NC, num_swdge_queues=NQ,
                   dynamic_dma_scratch_size=int(
                       os.environ.get("GCN_SCRATCH", str(16 * G))))

    xd_d = nc.dram_tensor("xd", [N, 2 * D0], f16, kind="ExternalInput")
    w1_d = nc.dram_tensor("W1", [D0, D1], f32, kind="ExternalInput")
    b1_d = nc.dram_tensor("b1", [1, D1], f32, kind="ExternalInput")
    w2_d = nc.dram_tensor("W2", [D1, D2], f32, kind="ExternalInput")
    b2_d = nc.dram_tensor("b2", [1, D2], f32, kind="ExternalInput")
    dis_act_d = nc.dram_tensor("dis_act", [WIN, NW], f32, kind="ExternalInput")
    disinv_d = nc.dram_tensor("disinv", [1, NW * WIN], f32, kind="ExternalInput")

    idx1_d, dst1_d, idx2_d, dst2_d = [], [], [], []
    for k in range(NSRC):
        T1, T2 = meta1["Tk"][k], meta2["Tk"][k]
        idx1_d.append(nc.dram_tensor(f"idx1_{k}", [128, T1 // 16], i16, kind="ExternalInput"))
        dst1_d.append(nc.dram_tensor(f"dst1_{k}", [128, T1 // 128], f16, kind="ExternalInput"))
        idx2_d.append(nc.dram_tensor(f"idx2_{k}", [128, T2 // 16], i16, kind="ExternalInput"))
        dst2_d.append(nc.dram_tensor(f"dst2_{k}", [128, T2 // 128], f16, kind="ExternalInput"))

    h_own = nc.dram_tensor("h_own", [NS, D1], f16, kind="Internal")
    hf = [nc.dram_tensor(f"hf{q}", [NC * Q4, D1], f16, kind="Internal",
                         addr_space="Shared") for q in range(NSRC)]
    hl = [nc.dram_tensor(f"hl{q}", [NC * Q4, D1], f16, kind="Internal")
          for q in range(NSRC)]
    if stage == "A":
        out_d = nc.dram_tensor("out", [D0, NW * WIN], f32, kind="ExternalOutput")
    elif stage == "AB":
        out_d = nc.dram_tensor("out", [NS, D1], f16, kind="ExternalOutput")
    elif stage == "ABC":
        out_d = nc.dram_tensor("out", [NC * Q4, D1], f16, kind="ExternalOutput")
    elif stage == "AD":
        out_d = nc.dram_tensor("out", [D1, NW * WIN], f32, kind="ExternalOutput")
    else:
        out_d = nc.dram_tensor("out", [NS, D2], f32, kind="ExternalOutput")
    acc2_dump = (nc.dram_tensor("acc2dump", [D1, NW * WIN], f32,
                                kind="ExternalOutput")
                 if stage == "full+dump" else None)
    if stage == "full+dump":
        stage = "full"

    # per-bucket chunk counts / prefix offsets per k-stream
    nch1 = np.zeros((NSRC, NW), dtype=np.int64)
    for k in range(NSRC):
        for (w, n) in meta1["segs"][k]:
            nch1[k][w] = n
    pre1 = np.zeros((NSRC, NW + 1), dtype=np.int64)
    for k in range(NSRC):
        pre1[k][1:] = np.cumsum(nch1[k])
    # first/last source-chunk contributing to each window (PSUM start/stop)
    fk = [min(k for k in range(NSRC) if nch1[k][w] > 0) for w in range(NW)]
    lk = [max(k for k in range(NSRC) if nch1[k][w] > 0) for w in range(NW)]

    qsems = [nc.alloc_semaphore(f"gq{i}") for i in range(NQ)]
    qstate = {"n": 0, "cum": [0] * NQ}

    def gather_call(gt_ap, src_ap, idx_ap, mlen, elem):
        """Issue one gather call.  With _PREP, descriptor generation is
        decoupled from the transfer (prepare_only + trigger) so the Pool
        engine never blocks on the DMA; the data-completion sync is the
        caller's job: we emit an explicit PE wait_ge on the queue's DMA
        semaphore right before the first consuming matmul (Tile's
        lane-sem waits are pre-bumped away by the framework)."""
        q = qstate["n"] % NQ
        qstate["n"] += 1
        if _PREP:
            nc.gpsimd.dma_gather(
                gt_ap, src_ap, idx_ap, mlen, mlen, elem,
                elem_step=src_ap.ap[0][0], queue_num=q,
                single_packet=_SINGLE_PACKET,
                prepare_only=True, sem=qsems[q])
            nc.gpsimd.trigger_dma(count=None, queue_num=q)
            qstate["cum"][q] += 16
            nc.tensor.wait_ge(qsems[q], qstate["cum"][q])
        else:
            nc.gpsimd.dma_gather(
                gt_ap, src_ap, idx_ap, mlen, mlen, elem,
                elem_step=src_ap.ap[0][0], queue_num=q,
                single_packet=_SINGLE_PACKET)

    with tile.TileContext(nc) as tc:
        with ExitStack() as top:
            const = top.enter_context(tc.tile_pool(name="const", bufs=1))
            w1_t = const.tile([D0, D1], f32)
            nc.sync.dma_start(w1_t[:], w1_d[:])
            b1_t = const.tile([1, D1], f32)
            nc.sync.dma_start(b1_t[:], b1_d[:])
            w2_t = const.tile([D1, D2], f32)
            nc.sync.dma_start(w2_t[:], w2_d[:])
            b2_t = const.tile([1, D2], f32)
            nc.sync.dma_start(b2_t[:], b2_d[:])
            dis_act_t = const.tile([WIN, NW], f32)
            nc.sync.dma_start(dis_act_t[:], dis_act_d[:])
            disinv_t = const.tile([1, NW * WIN], f32)
            nc.sync.dma_start(disinv_t[:], disinv_d[:])
            iota_t = const.tile([128, GC, WIN], f16)
            nc.gpsimd.iota(iota_t[:], pattern=[[0, GC], [1, WIN]], base=0,
                           channel_multiplier=0,
                           allow_small_or_imprecise_dtypes=True)

            accp = top.enter_context(tc.tile_pool(name="acc", bufs=1))
            acc1 = accp.tile([D0, NW * WIN], f32)
            nc.vector.memset(acc1[:], 0.0)
            acc2 = accp.tile([D1, NW * WIN], f32)
            nc.vector.memset(acc2[:], 0.0)

            xb = xd_d[:]
            x_aps = [bass.AP(xb.tensor, k * 2 * D0,
                             [[NSRC * 2 * D0, SC1], [1, 2 * D0]])
                     for k in range(NSRC)]

            def bcast(col_slice, mc):
                return bass.AP(col_slice.tensor, col_slice.offset,
                               [list(col_slice.ap[0]), [1, mc], [0, WIN]])

            # Deferred AllGather emission: the collective trigger sits in
            # the Pool queue and head-of-line blocks subsequent gather
            # calls while waiting on the h DMAs; emitting it AFTER the
            # next phase's first gather call hides that wait.
            pending_ag = []

            def flush_ag():
                while pending_ag:
                    qq = pending_ag.pop(0)
                    nc.gpsimd.collective_compute(
                        "AllGather", mybir.AluOpType.bypass,
                        replica_groups=[list(range(NC))],
                        ins=[h_own[qq * Q4:(qq + 1) * Q4, :]],
                        outs=[hf[qq][:, :]],
                    )
                    nc.sync.dma_start(hl[qq][:, :], hf[qq][:, :])

            # ---------------- Layer 1 + h + AllGather, per quarter --------
            with ExitStack() as l1s:
                mp = l1s.enter_context(tc.tile_pool(name="meta1", bufs=6))
                gp = l1s.enter_context(tc.tile_pool(name="g1", bufs=8))
                pp = l1s.enter_context(tc.tile_pool(name="p1", bufs=6))
                psp1 = l1s.enter_context(
                    tc.tile_pool(name="ps1", bufs=6, space="PSUM"))
                psb = l1s.enter_context(
                    tc.tile_pool(name="psb", bufs=2, space="PSUM"))
                hp = l1s.enter_context(tc.tile_pool(name="hb", bufs=4))

                for qi in range(4):
                    g0, g1 = WB[qi], WB[qi + 1]
                    for k in range(NSRC):
                        c0, c1 = int(pre1[k][g0]), int(pre1[k][g1])
                        total = c1 - c0
                        if total == 0:
                            continue
                        idx_t = mp.tile([128, total * 8], i16, tag="idx")
                        nc.sync.dma_start(
                            idx_t[:], idx1_d[k][:, c0 * 8:c1 * 8])
                        dst_t = mp.tile([128, total], f16, tag="dst")
                        nc.sync.dma_start(dst_t[:], dst1_d[k][:, c0:c1])
                        jj = 0
                        gt = None
                        P8 = None
                        for w in range(g0, g1):
                            nchk = int(nch1[k][w])
                            if nchk == 0:
                                continue
                            ps = psp1.tile([D0, WIN], f32, tag="ps1")
                            for j in range(nchk):
                                t, slot = divmod(jj, GC)
                                if slot == 0:
                                    mc = min(GC, total - t * GC)
                                    gt = gp.tile([128, GC, 2 * D0], f16,
                                                 tag="gt")
                                    gather_call(
                                        gt[:, :mc, :], x_aps[k],
                                        idx_t[:, t * GC * 8:
                                              t * GC * 8 + mc * 8],
                                        mc * 128, 2 * D0)
                                    flush_ag()
                                    P8 = pp.tile([128, GC, WIN], f16,
                                                 tag="P")
                                    nc.vector.tensor_tensor(
                                        P8[:, :mc, :], iota_t[:, :mc, :],
                                        bcast(dst_t[:, t * GC:t * GC + mc], mc),
                                        mybir.AluOpType.is_equal)
                                nc.tensor.matmul(
                                    ps[:], gt[:, slot, 0:D0],
                                    P8[:, slot, :],
                                    start=(j == 0), stop=(j == nchk - 1))
                                jj += 1
                            nc.vector.tensor_tensor(
                                acc1[:, w * WIN:(w + 1) * WIN],
                                acc1[:, w * WIN:(w + 1) * WIN], ps[:],
                                mybir.AluOpType.add)

                    # ---- h~ = dis * relu(dis*(acc1.T @ W1) + b1) ----
                    # ps = acc1.T@W1 + (1/dis)*b1; relu(ps*dis) applies the
                    # dst-side dis INSIDE the relu, the extra mul applies the
                    # src-side dis for layer-2 gathers.
                    for w in range(g0, g1):
                        M = min(WIN, NS - w * WIN)
                        ps = psb.tile([M, D1], f32, tag="psb")
                        nc.tensor.matmul(ps[:], acc1[:, w * WIN:w * WIN + M],
                                         w1_t[:], start=True, stop=False)
                        nc.tensor.matmul(
                            ps[:], disinv_t[:, w * WIN:w * WIN + M], b1_t[:],
                            start=False, stop=True)
                        ht = hp.tile([M, D1], f16, tag="ht")
                        nc.scalar.activation(ht[:], ps[:],
                                             mybir.ActivationFunctionType.Relu,
                                             scale=dis_act_t[0:M, w:w + 1])
                        ht2 = hp.tile([M, D1], f16, tag="ht2")
                        nc.scalar.mul(ht2[:], ht[:],
                                      dis_act_t[0:M, w:w + 1])
                        nc.sync.dma_start(h_own[w * WIN:w * WIN + M, :], ht2[:])

                    pending_ag.append(qi)

            if stage in ("A", "AB", "ABC"):
                flush_ag()
            if stage == "A":
                nc.sync.dma_start(out_d[:], acc1[:])
            elif stage == "AB":
                nc.sync.dma_start(out_d[:], h_own[:])
            elif stage == "ABC":
                nc.sync.dma_start(out_d[:], hl[0][:])

            # ------- Layer 2 (+ folded out-stage on the last k pass) -------
            with ExitStack() as l2s:
                if stage in ("A", "AB", "ABC"):
                    meta2 = {"Tk": [0] * NSRC, "segs": [[] for _ in range(NSRC)]}
                nch2 = np.zeros((NSRC, NW), dtype=np.int64)
                for k in range(NSRC):
                    for (w, n) in meta2["segs"][k]:
                        nch2[k][w] = n
                mp2 = l2s.enter_context(tc.tile_pool(name="meta2", bufs=2))
                gp2 = l2s.enter_context(tc.tile_pool(name="g2", bufs=8))
                pp2 = l2s.enter_context(tc.tile_pool(name="p2", bufs=6))
                psp = l2s.enter_context(
                    tc.tile_pool(name="ps2", bufs=6, space="PSUM"))
                op = l2s.enter_context(tc.tile_pool(name="ob", bufs=4))
                pso = l2s.enter_context(
                    tc.tile_pool(name="pso", bufs=2, space="PSUM"))

                for k in range(NSRC):
                    Tk = meta2["Tk"][k]
                    last_k = k == NSRC - 1
                    if Tk > 0:
                        total = Tk // 128
                        idx_t = mp2.tile([128, Tk // 16], i16, tag="idx2")
                        nc.sync.dma_start(idx_t[:], idx2_d[k][:])
                        dst_t = mp2.tile([128, total], f16, tag="dst2")
                        nc.sync.dma_start(dst_t[:], dst2_d[k][:])
                        hl_ap = hl[k][:]
                        jj = 0
                        gt = None
                        P8 = None
                    for w in range(NW):
                        nchk = int(nch2[k][w]) if Tk > 0 else 0
                        if nchk > 0:
                            ps = psp.tile([D1, WIN], f32, tag="ps2")
                            for j in range(nchk):
                                t, slot = divmod(jj, GC)
                                if slot == 0:
                                    mc = min(GC, total - t * GC)
                                    gt = gp2.tile([128, GC, D1], f16,
                                                  tag="gt2")
                                    gather_call(
                                        gt[:, :mc, :], hl_ap,
                                        idx_t[:, t * GC * 8:t * GC * 8 + mc * 8],
                                        mc * 128, D1)
                                    flush_ag()
                                    P8 = pp2.tile([128, GC, WIN], f16,
                                                  tag="P2")
                                    nc.vector.tensor_tensor(
                                        P8[:, :mc, :], iota_t[:, :mc, :],
                                        bcast(dst_t[:, t * GC:t * GC + mc], mc),
                                        mybir.AluOpType.is_equal)
                                nc.tensor.matmul(ps[:], gt[:, slot, :],
                                                 P8[:, slot, :],
                                                 start=(j == 0),
                                                 stop=(j == nchk - 1))
                                jj += 1
                            nc.vector.tensor_tensor(
                                acc2[:, w * WIN:(w + 1) * WIN],
                                acc2[:, w * WIN:(w + 1) * WIN], ps[:],
                                mybir.AluOpType.add)
                        if last_k and stage == "full":
                            # acc2[:, w] is final: out = dis*(acc2.T@W2) + b2
                            # emitted as dis*(acc2.T@W2 + (1/dis)*b2).
                            M = min(WIN, NS - w * WIN)
                            pso_t = pso.tile([M, D2], f32, tag="pso")
                            nc.tensor.matmul(
                                pso_t[:], acc2[:, w * WIN:w * WIN + M],
                                w2_t[:], start=True, stop=False)
                            nc.tensor.matmul(
                                pso_t[:], disinv_t[:, w * WIN:w * WIN + M],
                                b2_t[:], start=False, stop=True)
                            ot = op.tile([M, D2], f32, tag="ot")
                            nc.scalar.mul(ot[:], pso_t[:],
                                          dis_act_t[0:M, w:w + 1])
                            nc.sync.dma_start(
                                out_d[w * WIN:w * WIN + M, :], ot[:])

            if stage == "AD":
                nc.sync.dma_start(out_d[:], acc2[:])
            if acc2_dump is not None:
                nc.sync.dma_start(acc2_dump[:], acc2[:])

    nc.compile()
    return nc


# ----------------------------------------------------------------------------
# Entry point
# ----------------------------------------------------------------------------
def _ensure_axon_hooks_module():
    """bass_utils hard-imports antenv.axon_hooks when BASS_TRACE is set;
    provide a degradable stub if the image's antenv lacks it."""
    import types

    try:
        import antenv.axon_hooks  # noqa: F401
        return
    except ImportError:
        pass
    try:
        import antenv
    except ImportError:
        return
    mod = types.ModuleType("antenv.axon_hooks")
    mod._hook = None
    mod.set_axon_ntff_profile_hook = lambda h: setattr(mod, "_hook", h)
    mod.get_axon_ntff_profile_hook = lambda: mod._hook
    sys.modules["antenv.axon_hooks"] = mod
    antenv.axon_hooks = mod


def kernel(x, edge_index, W1, b1, W2, b2):
    _ensure_axon_hooks_module()
    from concourse import bass_utils

    meta1, meta2, in_maps = _preprocess(x, edge_index, W1, b1, W2, b2)
    nc = _build(meta1, meta2, debug=False)
    res = bass_utils.run_bass_kernel_spmd(nc, in_maps, core_ids=list(range(NC)))
    out = np.concatenate([r["out"] for r in res.results], axis=0)
    return out.astype(np.float32)

